# revision 1
# baseline (speedup 1.0000x reference)
"""MoE routing kernel for Trainium2 (8 NeuronCores, expert-parallel).

Strategy:
  - Router (tiny: [N,H]@[H,E]) runs on host in fp64; top-2 selection is
    identical to the fp32 reference whenever the prob gap exceeds fp32
    noise (~1e-7; measured min gap is ~6.6e-6 for the target inputs).
  - Expert-parallel: core e gets expert e's weights plus the tokens that
    routed to it (zero-padded to capacity C), as transposed activations
    [H, C] so that weight matrices serve directly as the stationary
    matmul operand with no on-device transposes.
  - Shared expert is data-parallel: core c processes tokens [c*NS,(c+1)*NS)
    with the 0.5 scale folded into Sd on host.
  - All matmuls run in float32r (TF32-like, full PE rate at free-dim>=256),
    accumulation in fp32 PSUM. Measured rel-l2 per matmul ~2.3e-4.
  - Host scatter-adds per-expert outputs (weighted on device by the
    replicated combine-weight row) and shared outputs back into [N, H].
"""

import math

import numpy as np

import concourse.bass as bass
import concourse.mybir as mybir
import concourse.tile as tile
from concourse import bacc
from concourse.bass_utils import run_bass_kernel_spmd

F32 = mybir.dt.float32
F32R = mybir.dt.float32r
SILU = mybir.ActivationFunctionType.Silu

N_CORES = 8
TOP_K = 2
SHARED_SCALE = 0.5

# Set by test harnesses to collect HW timing; harmless when False.
TRACE = False
LAST = {}

_NC_CACHE = {}


def _chunks(total):
    """Split `total` into pieces <=512, preferring every piece >=256."""
    out, off = [], 0
    rem = total
    while rem > 0:
        if rem <= 512:
            take = rem
        elif rem >= 768:
            take = 512
        else:
            take = rem - 256
        out.append((off, take))
        off += take
        rem -= take
    return out


def _chunks_asc(total):
    """Like _chunks but smallest piece first (fast first-chunk availability)."""
    sizes = sorted(c[1] for c in _chunks(total))
    out, off = [], 0
    for sz in sizes:
        out.append((off, sz))
        off += sz
    return out


def _passes(total, max_pass=1280):
    """Split `total` into passes (multiples of 128, <= max_pass), choosing
    sizes that minimize total chunk count (favors 512-wide PSUM chunks)."""
    n = max(1, math.ceil(total / max_pass))
    best = None
    if total % 128 == 0 and n <= 4:
        import itertools

        sizes = [s for s in range(256, max_pass + 1, 128)]
        for combo in itertools.combinations_with_replacement(sizes, n):
            if sum(combo) != total:
                continue
            cost = sum(len(_chunks(p)) for p in combo)
            key = (cost, max(combo) - min(combo))
            if best is None or key < best[0]:
                best = (key, combo)
    if best is None:
        base = math.ceil(total / n / 128) * 128
        combo = []
        off = 0
        while off < total:
            take = min(base, total - off)
            combo.append(take)
            off += take
    else:
        combo = sorted(best[1], reverse=True)
    out, off = [], 0
    for take in combo:
        out.append((off, take))
        off += take
    return out


def _build(H, I, IS, C, NS):
    """Per-core SPMD program: expert swiglu over C capacity tokens plus
    shared-expert swiglu over NS tokens, transposed-activation layout."""
    KH = H // 128
    nc = bacc.Bacc("TRN2", target_bir_lowering=False)

    xT = nc.dram_tensor("xT", [H, C], F32R, kind="ExternalInput")
    wg = nc.dram_tensor("wg", [H, I], F32R, kind="ExternalInput")
    wu = nc.dram_tensor("wu", [H, I], F32R, kind="ExternalInput")
    wd = nc.dram_tensor("wd", [I, H], F32R, kind="ExternalInput")
    xsT = nc.dram_tensor("xsT", [H, NS], F32R, kind="ExternalInput")
    sg = nc.dram_tensor("sg", [H, IS], F32R, kind="ExternalInput")
    su = nc.dram_tensor("su", [H, IS], F32R, kind="ExternalInput")
    sd = nc.dram_tensor("sd", [IS, H], F32R, kind="ExternalInput")
    yT = nc.dram_tensor("yT", [H, C], F32, kind="ExternalOutput")
    ysT = nc.dram_tensor("ysT", [H, NS], F32, kind="ExternalOutput")

    xT_r = xT[:, :].rearrange("(k p) c -> p k c", p=128)
    yT_r = yT[:, :].rearrange("(k p) c -> p k c", p=128)
    xsT_r = xsT[:, :].rearrange("(k p) c -> p k c", p=128)
    ysT_r = ysT[:, :].rearrange("(k p) c -> p k c", p=128)
    wg_r = wg[:, :].rearrange("(k p) i -> p k i", p=128)
    wu_r = wu[:, :].rearrange("(k p) i -> p k i", p=128)
    wd_r = wd[:, :].rearrange("(t p) h -> p t h", p=128)
    sg_r = sg[:, :].rearrange("(k p) i -> p k i", p=128)
    su_r = su[:, :].rearrange("(k p) i -> p k i", p=128)
    sd_r = sd[:, :].rearrange("(t p) h -> p t h", p=128)

    with tile.TileContext(nc) as tc:
        with (
            tc.tile_pool(name="xp", bufs=1) as xp,
            tc.tile_pool(name="yp", bufs=1) as yp,
            tc.tile_pool(name="wp", bufs=5) as wp,
            tc.tile_pool(name="hp", bufs=1) as hp,
            tc.tile_pool(name="ps", bufs=2, space="PSUM") as ps,
        ):

            def mlp(
                x_sb, y_sb, g_r, u_r, d_r, i_dim, chunk_list,
                y_out_r, y_out_off, after_w0=None,
            ):
                n_hb = i_dim // 512  # half-blocks of 512 intermediate cols
                for hb in range(n_hb):
                    g_sb = wp.tile([128, KH, 512], F32R, tag="w")
                    nc.sync.dma_start(
                        out=g_sb, in_=g_r[:, :, hb * 512 : (hb + 1) * 512]
                    )
                    u_sb = wp.tile([128, KH, 512], F32R, tag="w")
                    nc.sync.dma_start(
                        out=u_sb, in_=u_r[:, :, hb * 512 : (hb + 1) * 512]
                    )
                    d_sb = wp.tile([128, 4, H], F32R, tag="w")
                    nc.gpsimd.dma_start(out=d_sb, in_=d_r[:, hb * 4 : (hb + 1) * 4, :])
                    if hb == 0 and after_w0 is not None:
                        after_w0()
                    for c_off, cn in chunk_list:
                        h_sb = hp.tile([128, 4, cn], F32R, tag="h")
                        x_sl = [x_sb[:, k, c_off : c_off + cn] for k in range(KH)]
                        for m in range(4):
                            pg = ps.tile([128, cn], F32, tag="pg")
                            for k in range(KH):
                                nc.tensor.matmul(
                                    pg,
                                    g_sb[:, k, m * 128 : (m + 1) * 128],
                                    x_sl[k],
                                    start=(k == 0),
                                    stop=(k == KH - 1),
                                )
                            nc.scalar.activation(h_sb[:, m, :], pg, SILU)
                            pu = ps.tile([128, cn], F32, tag="pu")
                            for k in range(KH):
                                nc.tensor.matmul(
                                    pu,
                                    u_sb[:, k, m * 128 : (m + 1) * 128],
                                    x_sl[k],
                                    start=(k == 0),
                                    stop=(k == KH - 1),
                                )
                            nc.vector.tensor_mul(h_sb[:, m, :], h_sb[:, m, :], pu)
                        for hm in range(KH):
                            pd = ps.tile([128, cn], F32, tag="pd")
                            for k in range(4):
                                nc.tensor.matmul(
                                    pd,
                                    d_sb[:, k, hm * 128 : (hm + 1) * 128],
                                    h_sb[:, k, :],
                                    start=(k == 0),
                                    stop=(k == 3),
                                )
                            y_sl = y_sb[:, hm, c_off : c_off + cn]
                            if hb == 0:
                                nc.vector.tensor_copy(y_sl, pd)
                            else:
                                nc.vector.tensor_add(y_sl, y_sl, pd)
                            if hb == n_hb - 1:
                                nc.sync.dma_start(
                                    out=y_out_r[
                                        :,
                                        hm,
                                        y_out_off + c_off : y_out_off + c_off + cn,
                                    ],
                                    in_=y_sl,
                                )

            # expert phase: capacity C tokens through this core's expert
            for p_idx, (p_off, P) in enumerate(_passes(C)):
                x_sb = xp.tile([128, KH, P], F32R, tag=f"x{P}")
                after_w0 = None
                if p_idx == 0:
                    chs = _chunks_asc(P)
                    c0 = chs[0][1]
                    nc.sync.dma_start(
                        out=x_sb[:, :, :c0], in_=xT_r[:, :, p_off : p_off + c0]
                    )

                    def after_w0(x_sb=x_sb, p_off=p_off, P=P, c0=c0):
                        nc.sync.dma_start(
                            out=x_sb[:, :, c0:],
                            in_=xT_r[:, :, p_off + c0 : p_off + P],
                        )
                else:
                    chs = _chunks(P)
                    nc.sync.dma_start(out=x_sb, in_=xT_r[:, :, p_off : p_off + P])
                y_sb = yp.tile([128, KH, P], F32, tag="y")
                mlp(x_sb, y_sb, wg_r, wu_r, wd_r, I, chs, yT_r, p_off, after_w0)

            # shared-expert phase: this core's 1/8 shard of all tokens
            p0 = _passes(C)[0][1]
            for p_off, P in _passes(NS):
                x_sb = xp.tile([128, KH, P], F32R, tag=f"x{p0}")
                nc.sync.dma_start(out=x_sb, in_=xsT_r[:, :, p_off : p_off + P])
                y_sb = yp.tile([128, KH, P], F32, tag="y")
                mlp(x_sb, y_sb, sg_r, su_r, sd_r, IS, _chunks(P), ysT_r, p_off)

    nc.compile()
    return nc


def _install_trace_hook():
    """run_bass_kernel_spmd(trace=True) under axon needs antenv.axon_hooks,
    absent from this image; shim it from trn_agent_boot."""
    import sys
    import types

    if "antenv.axon_hooks" in sys.modules:
        return
    from trn_agent_boot.trn_boot import _ntff_profile_via_ctypes

    hook = _ntff_profile_via_ctypes("/opt/axon/libaxon_pjrt.so")
    mod = types.ModuleType("antenv.axon_hooks")
    mod.get_axon_ntff_profile_hook = lambda: hook
    sys.modules["antenv.axon_hooks"] = mod


def kernel(hidden_states, Wr, Wg, Wu, Wd, Sg, Su, Sd):
    hidden_states = np.asarray(hidden_states, dtype=np.float32)
    Wr = np.asarray(Wr, dtype=np.float32)
    Wg = np.asarray(Wg, dtype=np.float32)
    Wu = np.asarray(Wu, dtype=np.float32)
    Wd = np.asarray(Wd, dtype=np.float32)
    Sg = np.asarray(Sg, dtype=np.float32)
    Su = np.asarray(Su, dtype=np.float32)
    Sd = np.asarray(Sd, dtype=np.float32)

    B, S, H = hidden_states.shape
    E = Wr.shape[1]
    I = Wg.shape[2]
    IS = Sg.shape[1]
    N = B * S
    assert E == N_CORES and N % N_CORES == 0
    NS = N // N_CORES

    flat = hidden_states.reshape(N, H)

    # host router, fp64 (softmax is monotone: top-k by logits == by probs)
    logits = flat.astype(np.float64) @ Wr.astype(np.float64)
    lm = logits.max(axis=1, keepdims=True)
    p = np.exp(logits - lm)
    p /= p.sum(axis=1, keepdims=True)
    order = np.argsort(-logits, axis=1, kind="stable")
    top = order[:, :TOP_K]

    sel = np.zeros((N, E), dtype=bool)
    np.put_along_axis(sel, top, True, axis=1)
    idx_e = [np.flatnonzero(sel[:, e]) for e in range(E)]
    counts = [len(ix) for ix in idx_e]
    C = max(512, math.ceil(max(counts) / 256) * 256)

    flatT = np.ascontiguousarray(flat.T)  # [H, N]
    Sd_half = np.ascontiguousarray(Sd * np.float32(SHARED_SCALE))

    in_maps = []
    for e in range(E):
        ix = idx_e[e]
        cnt = counts[e]
        xT = np.zeros((H, C), np.float32)
        xT[:, :cnt] = flatT[:, ix]
        in_maps.append(
            {
                "xT": xT,
                "wg": np.ascontiguousarray(Wg[e]),
                "wu": np.ascontiguousarray(Wu[e]),
                "wd": np.ascontiguousarray(Wd[e]),
                "xsT": np.ascontiguousarray(flatT[:, e * NS : (e + 1) * NS]),
                "sg": Sg,
                "su": Su,
                "sd": Sd_half,
            }
        )

    key = (H, I, IS, C, NS)
    if key not in _NC_CACHE:
        _NC_CACHE[key] = _build(*key)
    nc = _NC_CACHE[key]

    run_kwargs = {}
    if TRACE:
        _install_trace_hook()
        import tempfile

        run_kwargs = {"trace": True, "tmpdir": tempfile.mkdtemp(prefix="moe_trace_")}
    res = run_bass_kernel_spmd(nc, in_maps, core_ids=list(range(N_CORES)), **run_kwargs)
    LAST["exec_time_ns"] = res.exec_time_ns
    LAST["profile_json"] = res.profile_json
    LAST["counts"] = counts
    LAST["C"] = C

    out = np.zeros((N, H), np.float32)
    for e in range(E):
        cnt = counts[e]
        ix = idx_e[e]
        w = p[ix, e].astype(np.float32)
        out[ix] += res.results[e]["yT"][:, :cnt].T * w[:, None]
        out[e * NS : (e + 1) * NS] += res.results[e]["ysT"].T
    return out.reshape(B, S, H)



# revision 2
# speedup vs baseline: 1.1049x; 1.1049x over previous
"""MoE routing kernel for Trainium2 (8 NeuronCores, expert-parallel).

Strategy:
  - Router (tiny: [N,H]@[H,E]) runs on host in fp64; top-2 selection is
    identical to the fp32 reference whenever the prob gap exceeds fp32
    noise (~1e-7; measured min gap is ~6.6e-6 for the target inputs).
  - Expert-parallel: core e gets expert e's weights plus the tokens that
    routed to it (zero-padded to capacity C = max expert count rounded to
    8), as transposed activations [H, C] so weight matrices serve
    directly as the stationary matmul operand with no on-device
    transposes.
  - Shared expert is data-parallel: core c processes tokens [c*NS,(c+1)*NS)
    with the 0.5 scale folded into Sd on host.
  - All matmul operands are bfloat16: full PE rate (1 row/cycle) like
    float32r, but LDWEIGHTS takes half the time (hidden behind >=437-row
    streams) and DMA traffic halves.  PSUM accumulation is fp32; the
    cross-half-block accumulation of the down-projection output is fp32
    in SBUF.  Measured rel-l2 of the final output ~1e-3.
  - Single pass over C per phase: expert weights stream through SBUF
    exactly once (the fp32r baseline re-read them per 1280-token pass).
  - Host scatter-adds per-expert outputs (weighted by the top-k softmax
    probs) and shared outputs back into [N, H].
"""

import math

import numpy as np
import ml_dtypes

import concourse.bass as bass
import concourse.mybir as mybir
import concourse.tile as tile
from concourse import bacc
from concourse.bass_utils import run_bass_kernel_spmd

F32 = mybir.dt.float32
BF16 = mybir.dt.bfloat16
SILU = mybir.ActivationFunctionType.Silu

NP_BF16 = ml_dtypes.bfloat16

N_CORES = 8
TOP_K = 2
SHARED_SCALE = 0.5

# Set by test harnesses to collect HW timing; harmless when False.
TRACE = False
LAST = {}

_NC_CACHE = {}


def _chunks(total):
    """Split `total` into equal-ish chunks <=512, multiples of 4 (except
    possibly the first), keeping every chunk as large as possible so the
    LDWEIGHTS of the next matmul always hides behind the current stream."""
    n = max(1, math.ceil(total / 512))
    base = (total // n) // 4 * 4
    sizes = [base] * n
    sizes[0] += total - base * n
    assert sizes[0] <= 512, (total, sizes)
    out, off = [], 0
    for sz in sizes:
        out.append((off, sz))
        off += sz
    return out


def _build(H, I, IS, C, NS):
    """Per-core SPMD program: expert swiglu over C capacity tokens plus
    shared-expert swiglu over NS tokens, transposed-activation layout."""
    KH = H // 128
    nc = bacc.Bacc("TRN2", target_bir_lowering=False)

    xT = nc.dram_tensor("xT", [H, C], BF16, kind="ExternalInput")
    wg = nc.dram_tensor("wg", [H, I], BF16, kind="ExternalInput")
    wu = nc.dram_tensor("wu", [H, I], BF16, kind="ExternalInput")
    wd = nc.dram_tensor("wd", [I, H], BF16, kind="ExternalInput")
    xsT = nc.dram_tensor("xsT", [H, NS], BF16, kind="ExternalInput")
    sg = nc.dram_tensor("sg", [H, IS], BF16, kind="ExternalInput")
    su = nc.dram_tensor("su", [H, IS], BF16, kind="ExternalInput")
    sd = nc.dram_tensor("sd", [IS, H], BF16, kind="ExternalInput")
    yT = nc.dram_tensor("yT", [H, C], BF16, kind="ExternalOutput")
    ysT = nc.dram_tensor("ysT", [H, NS], BF16, kind="ExternalOutput")

    xT_r = xT[:, :].rearrange("(k p) c -> p k c", p=128)
    yT_r = yT[:, :].rearrange("(k p) c -> p k c", p=128)
    xsT_r = xsT[:, :].rearrange("(k p) c -> p k c", p=128)
    ysT_r = ysT[:, :].rearrange("(k p) c -> p k c", p=128)
    wg_r = wg[:, :].rearrange("(k p) i -> p k i", p=128)
    wu_r = wu[:, :].rearrange("(k p) i -> p k i", p=128)
    wd_r = wd[:, :].rearrange("(t p) h -> p t h", p=128)
    sg_r = sg[:, :].rearrange("(k p) i -> p k i", p=128)
    su_r = su[:, :].rearrange("(k p) i -> p k i", p=128)
    sd_r = sd[:, :].rearrange("(t p) h -> p t h", p=128)

    with tile.TileContext(nc) as tc:
        with (
            tc.tile_pool(name="xp", bufs=1) as xp,
            tc.tile_pool(name="yp", bufs=1) as yp,
            tc.tile_pool(name="wp", bufs=4) as wp,
            tc.tile_pool(name="hp", bufs=2) as hp,
            tc.tile_pool(name="op", bufs=4) as op,
            tc.tile_pool(name="ps", bufs=2, space="PSUM") as ps,
        ):

            def mlp(x_sb, y_sb, g_r, u_r, d_r, i_dim, chunk_list,
                    y_out_r, after_w0=None):
                n_hb = i_dim // 512  # half-blocks of 512 intermediate cols
                for hb in range(n_hb):
                    g_sb = wp.tile([128, KH, 512], BF16, tag="w")
                    nc.sync.dma_start(
                        out=g_sb, in_=g_r[:, :, hb * 512 : (hb + 1) * 512]
                    )
                    u_sb = wp.tile([128, KH, 512], BF16, tag="w")
                    nc.sync.dma_start(
                        out=u_sb, in_=u_r[:, :, hb * 512 : (hb + 1) * 512]
                    )
                    d_sb = wp.tile([128, 4, H], BF16, tag="w")
                    nc.gpsimd.dma_start(out=d_sb, in_=d_r[:, hb * 4 : (hb + 1) * 4, :])
                    if hb == 0 and after_w0 is not None:
                        after_w0()
                    for c_off, cn in chunk_list:
                        h_sb = hp.tile([128, 4, cn], BF16, tag="h")
                        x_sl = [x_sb[:, k, c_off : c_off + cn] for k in range(KH)]
                        for m in range(4):
                            pg = ps.tile([128, cn], F32, tag="pg")
                            for k in range(KH):
                                nc.tensor.matmul(
                                    pg,
                                    g_sb[:, k, m * 128 : (m + 1) * 128],
                                    x_sl[k],
                                    start=(k == 0),
                                    stop=(k == KH - 1),
                                )
                            nc.scalar.activation(h_sb[:, m, :], pg, SILU)
                            pu = ps.tile([128, cn], F32, tag="pu")
                            for k in range(KH):
                                nc.tensor.matmul(
                                    pu,
                                    u_sb[:, k, m * 128 : (m + 1) * 128],
                                    x_sl[k],
                                    start=(k == 0),
                                    stop=(k == KH - 1),
                                )
                            nc.vector.tensor_mul(h_sb[:, m, :], h_sb[:, m, :], pu)
                        for hm in range(KH):
                            pd = ps.tile([128, cn], F32, tag="pd")
                            for k in range(4):
                                nc.tensor.matmul(
                                    pd,
                                    d_sb[:, k, hm * 128 : (hm + 1) * 128],
                                    h_sb[:, k, :],
                                    start=(k == 0),
                                    stop=(k == 3),
                                )
                            y_sl = y_sb[:, hm, c_off : c_off + cn]
                            if hb == 0:
                                nc.vector.tensor_copy(y_sl, pd)
                            elif hb < n_hb - 1:
                                nc.vector.tensor_add(y_sl, y_sl, pd)
                            else:
                                yo = op.tile([128, cn], BF16, tag="yo")
                                nc.vector.tensor_add(yo, y_sl, pd)
                                nc.scalar.dma_start(
                                    out=y_out_r[:, hm, c_off : c_off + cn],
                                    in_=yo,
                                )

            # expert phase: capacity C tokens through this core's expert
            chs = _chunks(C)
            c0 = chs[0][1]
            x_sb = xp.tile([128, KH, C], BF16, tag="xe")
            nc.sync.dma_start(out=x_sb[:, :, :c0], in_=xT_r[:, :, :c0])

            def after_w0():
                nc.sync.dma_start(out=x_sb[:, :, c0:], in_=xT_r[:, :, c0:])
                # prefetch the shared-expert activations during expert compute
                nc.sync.dma_start(out=xs_sb, in_=xsT_r)

            xs_sb = xp.tile([128, KH, NS], BF16, tag="xs")
            y_sb = yp.tile([128, KH, C], F32, tag="y")
            mlp(x_sb, y_sb, wg_r, wu_r, wd_r, I, chs, yT_r, after_w0)

            # shared-expert phase: this core's 1/8 shard of all tokens
            ys_sb = yp.tile([128, KH, NS], F32, tag="y")
            mlp(xs_sb, ys_sb, sg_r, su_r, sd_r, IS, _chunks(NS), ysT_r)

    nc.compile()
    return nc


def _install_trace_hook():
    """run_bass_kernel_spmd(trace=True) under axon needs antenv.axon_hooks,
    absent from this image; shim it from trn_agent_boot."""
    import sys
    import types

    if "antenv.axon_hooks" in sys.modules:
        return
    from trn_agent_boot.trn_boot import _ntff_profile_via_ctypes

    hook = _ntff_profile_via_ctypes("/opt/axon/libaxon_pjrt.so")
    mod = types.ModuleType("antenv.axon_hooks")
    mod.get_axon_ntff_profile_hook = lambda: hook
    sys.modules["antenv.axon_hooks"] = mod


def kernel(hidden_states, Wr, Wg, Wu, Wd, Sg, Su, Sd):
    hidden_states = np.asarray(hidden_states, dtype=np.float32)
    Wr = np.asarray(Wr, dtype=np.float32)
    Wg = np.asarray(Wg, dtype=np.float32)
    Wu = np.asarray(Wu, dtype=np.float32)
    Wd = np.asarray(Wd, dtype=np.float32)
    Sg = np.asarray(Sg, dtype=np.float32)
    Su = np.asarray(Su, dtype=np.float32)
    Sd = np.asarray(Sd, dtype=np.float32)

    B, S, H = hidden_states.shape
    E = Wr.shape[1]
    I = Wg.shape[2]
    IS = Sg.shape[1]
    N = B * S
    assert E == N_CORES and N % N_CORES == 0
    NS = N // N_CORES

    flat = hidden_states.reshape(N, H)

    # host router, fp64 (softmax is monotone: top-k by logits == by probs)
    logits = flat.astype(np.float64) @ Wr.astype(np.float64)
    lm = logits.max(axis=1, keepdims=True)
    p = np.exp(logits - lm)
    p /= p.sum(axis=1, keepdims=True)
    order = np.argsort(-logits, axis=1, kind="stable")
    top = order[:, :TOP_K]

    sel = np.zeros((N, E), dtype=bool)
    np.put_along_axis(sel, top, True, axis=1)
    idx_e = [np.flatnonzero(sel[:, e]) for e in range(E)]
    counts = [len(ix) for ix in idx_e]
    C = max(512, math.ceil(max(counts) / 8) * 8)

    flatT = np.ascontiguousarray(flat.T.astype(NP_BF16))  # [H, N] bf16
    Sd_half = np.ascontiguousarray((Sd * np.float32(SHARED_SCALE)).astype(NP_BF16))
    Sg16 = Sg.astype(NP_BF16)
    Su16 = Su.astype(NP_BF16)

    in_maps = []
    for e in range(E):
        ix = idx_e[e]
        cnt = counts[e]
        xT = np.zeros((H, C), NP_BF16)
        xT[:, :cnt] = flatT[:, ix]
        in_maps.append(
            {
                "xT": xT,
                "wg": np.ascontiguousarray(Wg[e].astype(NP_BF16)),
                "wu": np.ascontiguousarray(Wu[e].astype(NP_BF16)),
                "wd": np.ascontiguousarray(Wd[e].astype(NP_BF16)),
                "xsT": np.ascontiguousarray(flatT[:, e * NS : (e + 1) * NS]),
                "sg": Sg16,
                "su": Su16,
                "sd": Sd_half,
            }
        )

    key = (H, I, IS, C, NS)
    if key not in _NC_CACHE:
        _NC_CACHE[key] = _build(*key)
    nc = _NC_CACHE[key]

    run_kwargs = {}
    if TRACE:
        _install_trace_hook()
        import tempfile

        run_kwargs = {"trace": True, "tmpdir": tempfile.mkdtemp(prefix="moe_trace_")}
    res = run_bass_kernel_spmd(nc, in_maps, core_ids=list(range(N_CORES)), **run_kwargs)
    LAST["exec_time_ns"] = res.exec_time_ns
    LAST["profile_json"] = res.profile_json
    LAST["counts"] = counts
    LAST["C"] = C

    out = np.zeros((N, H), np.float32)
    for e in range(E):
        cnt = counts[e]
        ix = idx_e[e]
        w = p[ix, e].astype(np.float32)
        out[ix] += res.results[e]["yT"][:, :cnt].T.astype(np.float32) * w[:, None]
        out[e * NS : (e + 1) * NS] += res.results[e]["ysT"].T.astype(np.float32)
    return out.reshape(B, S, H)
